# revision 22
# baseline (speedup 1.0000x reference)
"""Trainium2 Bass kernel for char-CNN: 5-tap conv along word_length + max-pool.

Reference computation (per (batch, sentence) word, shapes B=64 S=256 W=20 E=128):
    y[w, e] = sum_{kh=0..4} x[w + kh - 2, e] * conv_w[kh]     (zero padded)
    out[e]  = max_w y[w, e] + conv_b

Strategy (v2):
  - Data-parallel over 8 NeuronCores: 8 batches (2048 words) per core.
  - Host packs each core's shard as fp8 E3M4 (4 mantissa bits; measured
    output rel err 9.7e-3 vs 2e-2 budget) in z[(j w)=120, group, e=128]
    layout (J=6 words per group).  fp8 quarters HBM traffic vs f32.
  - Conv as banded matmul on TensorE: stationary lhsT = x [120, 128] fp8
    (fast-weight-load eligible), moving rhs = block-diagonal A [120, 120]
    fp16 (kept high precision; mixed-dtype MM is allowed), PSUM f32
    out [e=128, (wo, j)=120] per group, 16 groups per 4-bank PSUM chunk.
  - PSUM evacuation = the max reduction, split across engines per-chunk:
      'D': DVE tensor_max pairs straight out of PSUM (2 f32/cycle) ->
           f16, then DVE f16 2x tail tree.
      'A': ACT copies PSUM -> SBUF f16, DVE does the full f16 tree.
      'G': like 'D' but the f16 tail tree runs on GpSimd.
    The pattern is tunable to balance DVE/ACT/GPSIMD occupancy.
  - Input DMA: few large chunks on the HWDGE (sync) ring.
"""

from contextlib import ExitStack

import numpy as np
import ml_dtypes

import concourse.bass as bass
import concourse.mybir as mybir
import concourse.tile as tile
from concourse import bacc

W = 20  # word length
E = 128  # embedding dim
KH = 5  # conv taps
PAD = 2
J = 6  # words per matmul group (6 * 20 = 120 <= 128 partitions)
KP = J * W  # contraction size / partitions used (120)
CG = 16  # groups per compute sub-chunk (4 PSUM banks)
NCORES = 8
BANK = 512  # PSUM bank size in f32 elements

# --- tuning knobs ---
DMA_SIZES = (16, 48, 96, 96, 96)  # input chunk sizes in groups (sum = NGP)
DMA_RINGS = ("scalar",)  # per chunk
EVAC_PATTERN = "RAAAA"  # cycled over compute sub-chunks (R/A)
TAIL_R = 1  # force R-mode on the last TAIL_R sub-chunks (short critical path)
TREE_BATCH = 4  # batch tree levels 2-5 across this many A-chunks
A_DTYPE = "float16"  # conv matrix dtype
FLUSH_EVERY = 6  # output flush period in sub-chunks


def build_conv_matrix(conv_w: np.ndarray) -> np.ndarray:
    """[KP, KP] conv matrix, output columns wo-major / j-minor:
    A[j*W + wi, wo*J + j] = conv_w[wi - wo + 2].  With j innermost
    (step-1 runs of 6) every max-tree level gets DVE 2x f16 mode."""
    wv = np.asarray(conv_w, np.float32).reshape(-1)
    assert wv.shape == (KH,)
    a = np.zeros((KP, KP), np.float32)
    for j in range(J):
        for wo in range(W):
            for kh in range(KH):
                wi = wo + kh - PAD
                if 0 <= wi < W:
                    a[j * W + wi, wo * J + j] = wv[kh]
    return a.astype(ml_dtypes.bfloat16 if A_DTYPE == "bfloat16" else np.float16)


def pack_input(x_core: np.ndarray, ngp: int) -> np.ndarray:
    """[nw, W, E] f32 -> [KP, ngp, E] fp8-e3m4 partition-major, zero-padded
    to ngp*J words."""
    nw = x_core.shape[0]
    xp = np.zeros((ngp * J, W, E), np.float32)
    xp[:nw] = x_core
    # (g j) w e -> (j w) g e
    z = np.ascontiguousarray(
        xp.reshape(ngp, J, W, E).transpose(1, 2, 0, 3).reshape(KP, ngp, E)
    )
    return z.astype(ml_dtypes.float8_e3m4)


def build_nc(
    nw: int,
    dma_sizes=DMA_SIZES,
    dma_rings=DMA_RINGS,
    evac_pattern=EVAC_PATTERN,
    flush_every=FLUSH_EVERY,
) -> bass.Bass:
    """Build the per-core Bass graph. nw = real words per core."""
    f32 = mybir.dt.float32
    f16 = mybir.dt.float16
    bf16 = mybir.dt.bfloat16
    f8 = mybir.dt.float8e3
    a_dt = bf16 if A_DTYPE == "bfloat16" else f16

    ngp = sum(dma_sizes)  # padded group count (multiple of CG)
    assert ngp % CG == 0 and ngp * J >= nw
    nchunks = ngp // CG
    HW_ = W // 2  # 10

    nc = bacc.Bacc()
    z_ext = nc.declare_dram_parameter("z", [KP, ngp, E], f8, isOutput=False)
    a_ext = nc.declare_dram_parameter("a", [KP, KP], a_dt, isOutput=False)
    out_ext = nc.declare_dram_parameter("out", [E, nw], f16, isOutput=True)

    engines = {"sync": nc.sync, "scalar": nc.scalar, "gpsimd": nc.gpsimd}

    with ExitStack() as ctx:
        tc = ctx.enter_context(tile.TileContext(nc))
        const = ctx.enter_context(tc.tile_pool(name="const", bufs=1))
        hpool = ctx.enter_context(tc.tile_pool(name="xh", bufs=1))
        opool = ctx.enter_context(tc.tile_pool(name="o", bufs=1))
        spool = ctx.enter_context(tc.tile_pool(name="ys", bufs=3))
        u1pool = ctx.enter_context(tc.tile_pool(name="u1", bufs=3))
        u2pool = ctx.enter_context(tc.tile_pool(name="u2", bufs=3))
        u3pool = ctx.enter_context(tc.tile_pool(name="u3", bufs=3))
        u4pool = ctx.enter_context(tc.tile_pool(name="u4", bufs=3))
        pspool = ctx.enter_context(tc.tile_pool(name="ps", bufs=2, space="PSUM"))

        a_t = const.tile([KP, KP], a_dt)
        nc.sync.dma_start(out=a_t[:, :], in_=a_ext[:, :])
        maxt = opool.tile([E, ngp * J], f16)

        # --- phase A: issue the whole input stream up front ---
        chunks = []  # (xh_tile, base_group_of_tile)
        g0 = 0
        for i, gn in enumerate(dma_sizes):
            eng = engines[dma_rings[i % len(dma_rings)]]
            src = z_ext[:, g0 : g0 + gn, :].rearrange("p g e -> p (g e)")
            xh = hpool.tile([KP, gn * E], f8, tag=f"xh{i}")
            eng.dma_start(out=xh[:, :], in_=src)
            chunks.append((xh, g0, gn))
            g0 += gn

        def sub_sources():
            """Yield (xh, col_off, sg0) per CG-group compute sub-chunk."""
            for xh, base, gn in chunks:
                for s0 in range(0, gn, CG):
                    yield xh, s0 * E, base + s0

        # --- per-chunk compute ---
        def do_matmuls(xh, coff):
            ps = pspool.tile([E, 4 * BANK], f32, tag="ps")
            for g in range(CG):
                col = (g // 4) * BANK + (g % 4) * KP
                nc.tensor.matmul(
                    ps[:, col : col + KP],
                    lhsT=xh[:, coff + g * E : coff + (g + 1) * E],
                    rhs=a_t[:, :],
                    start=True,
                    stop=True,
                )
            return ps

        def ps_view(ps):
            """PSUM as [p, k(4), s(4), j(6), wo(20, stride J)] for reduce."""
            return (
                ps[:, :]
                .rearrange("p (k x) -> p k x", k=4)[:, :, 0 : 4 * KP]
                .rearrange("p k (s wo j) -> p k s j wo", wo=W, j=J)
            )

        def act_copy(ps):
            """ACT copies the PSUM chunk to SBUF f16 in two half-chunk ops
            (first half can start while the second half's MMs still run)."""
            s16 = spool.tile([E, CG * KP], f16, tag="s16")
            for h in range(2):
                nc.scalar.copy(
                    s16[:, h * 2 * KP * 4 : (h + 1) * 2 * KP * 4].rearrange(
                        "p (k x) -> p k x", k=2
                    ),
                    ps[:, h * 2 * BANK : (h + 1) * 2 * BANK].rearrange(
                        "p (k x) -> p k x", k=2
                    )[:, :, 0 : 4 * KP],
                )
            return s16.rearrange("p (k s wo j) -> p k s wo j", s=4, wo=W, j=J)

        def tail_tree(u1cat, nb, sg0_first):
            """Batched max-tree levels 2-5 over nb consecutive A-chunks.
            u1cat: [E, nb*CG*HW_*J] f16, viewed [p, c=(nb*16), wo(10), j]."""
            C = nb * CG
            u1v = u1cat[:, 0 : C * HW_ * J].rearrange(
                "p (c wo j) -> p c wo j", c=C, j=J
            )
            u2 = u2pool.tile([E, TREE_BATCH * CG * 5 * J], f16, tag="u2")
            u2v = u2[:, 0 : C * 5 * J].rearrange("p (c wo j) -> p c wo j", c=C, j=J)
            nc.vector.tensor_max(u2v, u1v[:, :, 0:5, :], u1v[:, :, 5:10, :])
            u3 = u3pool.tile([E, TREE_BATCH * CG * 2 * J], f16, tag="u3")
            u3v = u3[:, 0 : C * 2 * J].rearrange("p (c wo j) -> p c wo j", c=C, j=J)
            nc.vector.tensor_max(u3v, u2v[:, :, 0:2, :], u2v[:, :, 2:4, :])
            u4 = u4pool.tile([E, TREE_BATCH * CG * J], f16, tag="u4")
            u4v = u4[:, 0 : C * J].rearrange("p (c wo j) -> p c wo j", c=C, j=J)
            nc.vector.tensor_max(u4v, u3v[:, :, 0:1, :], u3v[:, :, 1:2, :])
            nc.vector.tensor_max(
                maxt[:, sg0_first * J : (sg0_first + C) * J].rearrange(
                    "p (c wo j) -> p c wo j", c=C, j=J
                ),
                u4v,
                u2v[:, :, 4:5, :],
            )

        w_flushed = 0

        def flush_out(upto_words, force):
            nonlocal w_flushed
            hi = min(upto_words, nw)
            if hi > w_flushed and (force or hi - w_flushed >= 384):
                nc.sync.dma_start(
                    out=out_ext[:, w_flushed:hi], in_=maxt[:, w_flushed:hi]
                )
                w_flushed = hi

        # pending batched-A state
        u1cat = None
        pend_n = 0
        pend_sg0 = 0

        def close_batch():
            nonlocal u1cat, pend_n
            if pend_n:
                tail_tree(u1cat, pend_n, pend_sg0)
                u1cat = None
                pend_n = 0

        for idx, (xh, coff, sg0) in enumerate(sub_sources()):
            kind = evac_pattern[idx % len(evac_pattern)]
            if idx >= nchunks - TAIL_R:
                kind = "R"
            ps = do_matmuls(xh, coff)
            if kind == "R":
                close_batch()
                # one DVE reduce_max straight out of PSUM (1 f32/cycle)
                nc.vector.reduce_max(
                    maxt[:, sg0 * J : (sg0 + CG) * J].rearrange(
                        "p (k s j) -> p k s j", k=4, s=4
                    ),
                    ps_view(ps),
                    axis=mybir.AxisListType.X,
                )
            elif kind == "A":
                # ACT copy -> f16; DVE level-1 into the batch tile; levels
                # 2-5 run once per TREE_BATCH consecutive A-chunks
                sv = act_copy(ps)
                if pend_n == 0:
                    u1cat = u1pool.tile([E, TREE_BATCH * CG * HW_ * J], f16, tag="u1")
                    pend_sg0 = sg0
                seg = CG * HW_ * J
                u1v = u1cat[:, pend_n * seg : (pend_n + 1) * seg].rearrange(
                    "p (k s wo j) -> p k s wo j", k=4, s=4, j=J
                )
                nc.vector.tensor_max(
                    u1v, sv[:, :, :, 0:HW_, :], sv[:, :, :, HW_:W, :]
                )
                pend_n += 1
                if pend_n == TREE_BATCH:
                    close_batch()
            else:
                raise ValueError(f"unknown evac kind {kind}")
            done = (sg0 + CG) * J if (kind == "R" or pend_n == 0) else sg0 * J
            flush_out(done, force=False)
            if idx == nchunks - 1:
                close_batch()
                flush_out(nw, force=True)
    nc.finalize()
    return nc


def _prep(x: np.ndarray, conv_w: np.ndarray):
    B, S, Wl, El = x.shape
    bs = B // NCORES
    nw = bs * S
    ngp = sum(DMA_SIZES)
    a_m = build_conv_matrix(conv_w)
    in_maps = [
        {
            "z": pack_input(x[i * bs : (i + 1) * bs].reshape(nw, Wl, El), ngp),
            "a": a_m,
        }
        for i in range(NCORES)
    ]
    return bs, nw, in_maps


def kernel(embedded_char, conv_w, conv_b):
    from concourse.bass_utils import run_bass_kernel_spmd

    x = np.asarray(embedded_char, np.float32)
    b_val = float(np.asarray(conv_b, np.float32).reshape(-1)[0])
    B, S, Wl, El = x.shape
    assert (Wl, El) == (W, E)
    bs, nw, in_maps = _prep(x, conv_w)

    nc = build_nc(nw)
    res = run_bass_kernel_spmd(nc, in_maps, core_ids=list(range(NCORES)))
    full = np.concatenate(
        [r["out"].astype(np.float32).T.reshape(bs, S, El) for r in res.results],
        axis=0,
    )
    if b_val != 0.0:
        full = full + b_val
    return np.ascontiguousarray(full.astype(np.float32))
